# revision 14
# baseline (speedup 1.0000x reference)
"""FP8StaticLinear Trainium2 kernel.

out = requant_fp8(qdq_fp8(x, s_in) @ (w * s_w).T + bias, s_out)

Sharding: data-parallel over tokens (B*S=16384 -> 2048/core on 8 cores).
Device math: fp8e4 DoubleRow matmuls on the PE array. Both operands are
halved on entry so the OCP-e4m3fn grid (max 448) maps onto TRN fp8e4
(max 240) exactly; scales are folded into the epilogue and the host-side
pre/post conversions (x is pre-quantized to fp8 on the host, the output
leaves the device as fp8 and is expanded to f32*2os on the host — both
are exact given the static per-tensor scales).
"""

import numpy as np
import ml_dtypes

import concourse.bass as bass
import concourse.mybir as mybir
from concourse.tile import TileContext
from concourse.vector_clock import ScopedClock
from concourse.bass_utils import run_bass_kernel_spmd

FP8 = mybir.dt.float8e4
F32 = mybir.dt.float32
NP_FP8 = ml_dtypes.float8_e4m3  # TRN fp8e4 (max 240, has inf)

N_CORES = 8
P = 128


# ---------------------------------------------------------------------------
# Workaround: this walrus build rejects >1 sem-wait on the Tile tail Drain
# ("Too many sync wait commands"). Split the waits across single-wait drains.
def _drain_and_barrier(self, tick_clock, wait_clock):
    drain_inst = self.nc.sync.drain()
    wait_clock.add_sem_waits(
        drain_inst.ins, ScopedClock({None: tick_clock.global_clock})
    )
    w = list(drain_inst.ins.sync_info.on_wait)
    if len(w) > 1:
        drain_inst.ins.sync_info = mybir.SyncInfo(on_wait=[w[0]], on_update=[])
        for extra in w[1:]:
            d2 = self.nc.sync.drain()
            d2.ins.sync_info = mybir.SyncInfo(on_wait=[extra], on_update=[])
    self.nc.all_engine_barrier()
    assert self.sems is not None
    popped = self.nc._tile_sem_poison_stack.pop()
    assert popped is self._sem_poison
    self.nc.clear_and_free_semaphores(list(self.sems.allocated().values()))
    self.nc.all_engine_barrier()


TileContext._drain_and_barrier = _drain_and_barrier


def split_sync_waits(nc, max_waits=1):
    """Hoist extra sem-waits onto standalone EventSemaphore carriers.

    This walrus build's setupSyncWait rejects instructions carrying more
    than one sem-wait ("Too many sync wait commands"), so any instruction
    with N>1 waits becomes N-1 single-wait EventSemaphore instructions on
    the same engine followed by the original instruction with one wait.
    """
    n_new = 0
    for f in nc.m.functions:
        for blk in f.blocks:
            out = []
            changed = False
            for inst in blk.instructions:
                si = inst.sync_info
                w = list(si.on_wait) if si is not None else []
                if len(w) > max_waits:
                    upd = list(inst.sync_info.on_update)
                    for wi in w[max_waits:]:
                        es = mybir.InstEventSemaphore(
                            name=f"hoistw-{n_new}", ins=[], outs=[]
                        )
                        n_new += 1
                        es.engine = inst.engine
                        es.sync_info = mybir.SyncInfo(on_wait=[wi], on_update=[])
                        out.append(es)
                    inst.sync_info = mybir.SyncInfo(
                        on_wait=w[:max_waits], on_update=upd
                    )
                    changed = True
                out.append(inst)
            if changed:
                blk.instructions = out
    return nc
# ---------------------------------------------------------------------------


def build(K, M, N, MF=512, warm_mms=28):
    """One-core program: out8 = fp8 requant of (qx @ w.T)*alpha + bias2.

    DRAM inputs:
      qxt    [MB, 128, KS, MF] fp8   host-quantized x shard:
                     qxt[mb, p, j, m] = fp8(x[mb*MF+m, j*128+p] / (2*s_in))
      wt     [NT, 128, KS, 128] fp8  halved weight, tiled:
                     wt[nt, p, j, n] = fp8(w[nt*128+n, j*128+p] / 2)
      bias2  [N]  f32   bias / (2*s_out)
      alpha  [1, 1] f32  2*s_in*s_w/s_out
    Output:
      out8   [MB, NT, 128, MF] fp8   out8[mb, nt, p, m] = fp8(clamp(
                     (x@w.T + b)[mb*MF+m, nt*128+p] / (2*s_out), +-224))
    """
    KS = K // P          # k subtiles of 128
    JP = KS // 2         # DoubleRow pairs
    NT = N // P          # n tiles
    MB = M // MF         # m blocks
    AF = mybir.ActivationFunctionType
    OP = mybir.AluOpType

    nc = bass.Bass()
    qxt = nc.dram_tensor("qxt", [MB, P, KS, MF], FP8, kind="ExternalInput")
    wt = nc.dram_tensor("wt", [NT, P, KS, P], FP8, kind="ExternalInput")
    bias2_d = nc.dram_tensor("bias2", [P, N // P], F32, kind="ExternalInput")
    alpha_d = nc.dram_tensor("alpha", [P, 1], F32, kind="ExternalInput")
    out8 = nc.dram_tensor("out8", [MB, NT, P, MF], FP8, kind="ExternalOutput")

    with TileContext(nc) as tc:
        with (
            tc.tile_pool(name="consts", bufs=1) as consts,
            tc.tile_pool(name="wres", bufs=1) as wres,
            tc.tile_pool(name="qx", bufs=3) as qxp,
            tc.tile_pool(name="psum", bufs=8, space="PSUM") as psp,
            tc.tile_pool(name="epi", bufs=3) as epi,
            tc.tile_pool(name="q8", bufs=3) as q8p,
        ):
            # ---- PE clock warm-up: the HAM gate releases the 2.4 GHz
            # clock only after ~3.4us of sustained PE activity. Burn that
            # window on dummy matmuls while the first real weight/x DMAs
            # are still in flight, so the real stream starts at full rate.
            # The dummy operand tile is left almost entirely uninitialized
            # -- a full init (memset or DMA) would push the warm-up past
            # the point where real data lands. Tile only requires at least
            # one write to materialize the tile, so memset a single
            # element; the rest is garbage fp8, which is fine since the
            # target PSUM bank is overwritten (start=True) before its
            # first real use.
            zd = consts.tile([P, 2, MF], FP8)
            nc.gpsimd.memset(zd[:, 0:1, 0:1], 0.0)
            ps_dummy = psp.tile([P, MF], F32, tag="ps", name="psdummy")
            for _ in range(warm_mms):
                nc.tensor.matmul(
                    ps_dummy[:, 0:256], zd[:, :, 0:P], zd[:, :, 0:256],
                    start=True, stop=True,
                    perf_mode=mybir.MatmulPerfMode.DoubleRow,
                )

            # ---- weights: resident, one tile per nt. Warm-group tiles
            # stream in k-quarters ordered the way the k-outer warm loop
            # consumes them; the rest trail in halves. All weight + later
            # x-block DMAs ride the gpsimd SWDGE queue so the sync queue
            # stays clear for block-0 chunks and output stores.
            w_tiles = []

            def alloc_w(nt):
                w_nt = wres.tile([P, KS, P], FP8, tag=f"w{nt}", name=f"w{nt}")
                w_tiles.append(w_nt)
                return w_nt

            # constants FIRST on the queue: Tile waits on per-HW-queue DMA
            # completion counters in program order, so a late-emitted const
            # DMA would make every epilogue wait for all earlier DMAs
            alpha = consts.tile([P, 1], F32)
            nc.sync.dma_start(alpha[:], alpha_d[:, :])
            # bias2[p, nt] = bias[nt*128+p] / (2*os), transposed on host so
            # the DMA is contiguous (a strided rearrange costs ~4us of
            # descriptor generation on the critical path)
            bias2 = consts.tile([P, NT], F32)
            nc.sync.dma_start(bias2[:], bias2_d[:, :])

            # NW=7 warm groups + the dummy tile fill all 8 PSUM banks, and
            # the first steady group (nt=7) lands on the dummy bank, whose
            # matmuls finished long ago -- no epilogue wait at the
            # warm->steady boundary
            NW = min(7, NT)
            for g in range(NW):
                alloc_w(g)
            # block-0 x chunks on the sync queue (highest priority: the
            # PE is gated on the first two)
            qx_tiles = {}
            for mb in range(MB):
                qx_tiles[mb] = qxp.tile([P, KS, MF], FP8, tag="qx", name=f"qx{mb}")
            # block-0 x loads in quarter-blocks of 8 chunks (512 KB,
            # contiguous 4KB per-partition lines -> one cheap issue each);
            # the warm loop consumes 4 jj-rows per quarter, slower than
            # delivery
            for j0 in range(0, KS, 8):
                nc.sync.dma_start(
                    qx_tiles[0][:, j0 : j0 + 8, :],
                    qxt[0, :, j0 : j0 + 8, :],
                )
            # warm tiles in halves, first halves of all tiles first: the
            # opening warm row then needs ~2.3MB in flight instead of 4MB
            h = KS // 2
            for g in range(NW):
                nc.gpsimd.dma_start(w_tiles[g][:, :h, :], wt[g, :, :h, :])
            for g in range(NW):
                nc.gpsimd.dma_start(w_tiles[g][:, h:, :], wt[g, :, h:, :])
            for nt in range(NW, NT):
                w_nt = alloc_w(nt)
                nc.gpsimd.dma_start(w_nt[:], wt[nt, :, :, :])
            # later x blocks trail on the same queue (pool WAR sems hold
            # them until the buffer frees; plenty of slack)
            CG = 8  # chunks per DMA for the later blocks
            for mb in range(1, MB):
                for j0 in range(0, KS, CG):
                    nc.gpsimd.dma_start(
                        qx_tiles[mb][:, j0 : j0 + CG, :],
                        qxt[mb, :, j0 : j0 + CG, :],
                    )

            def emit_mms(ps, nt, qx):
                for jj in range(JP):
                    nc.tensor.matmul(
                        ps[:],
                        w_tiles[nt][:, 2 * jj : 2 * jj + 2, :],
                        qx[:, 2 * jj : 2 * jj + 2, :],
                        start=(jj == 0),
                        stop=(jj == JP - 1),
                        perf_mode=mybir.MatmulPerfMode.DoubleRow,
                    )

            # epilogue: t = ps*alpha + bias/(2os); out8 = fp8(clamp t)
            def emit_epilogue(ps, nt, mb):
                t = epi.tile([P, MF], F32, tag="t", name="t")
                nc.scalar.activation(
                    t[:], ps[:], AF.Identity,
                    bias=bias2[:, nt : nt + 1], scale=alpha[:, 0:1],
                )
                q8 = q8p.tile([P, MF], FP8, tag="q8", name="q8")
                nc.vector.tensor_scalar(
                    q8[:], t[:], -224.0, 224.0, OP.max, OP.min
                )
                nc.sync.dma_start(out8[mb, nt, :, :], q8[:])

            # ---- main loop over m blocks ----
            for mb in range(MB):
                qx = qx_tiles[mb]
                if mb == 0:
                    # warm-up: first NW groups accumulate k-outer across NW
                    # psum banks, so the PE issues NW matmuls per arriving
                    # chunk pair instead of idling for the full qx0
                    ps_warm = [
                        psp.tile([P, MF], F32, tag="ps", name=f"psw{g}")
                        for g in range(NW)
                    ]
                    for jj in range(JP):
                        for g in range(NW):
                            nc.tensor.matmul(
                                ps_warm[g][:],
                                w_tiles[g][:, 2 * jj : 2 * jj + 2, :],
                                qx[:, 2 * jj : 2 * jj + 2, :],
                                start=(jj == 0),
                                stop=(jj == JP - 1),
                                perf_mode=mybir.MatmulPerfMode.DoubleRow,
                            )
                    for g in range(NW):
                        emit_epilogue(ps_warm[g], g, mb)
                    nt_range = list(range(NW, NT))
                else:
                    nt_range = list(range(NT))

                for nt in nt_range:
                    ps = psp.tile([P, MF], F32, tag="ps", name="ps")
                    emit_mms(ps, nt, qx)
                    emit_epilogue(ps, nt, mb)
    return split_sync_waits(nc)


def prep_weight(weight):
    """[N, K] f32 (e4m3fn-grid values) -> [NT, 128, KS, 128] TRN-fp8 of w/2."""
    N, K = weight.shape
    wq = (weight.astype(np.float32) * 0.5).astype(NP_FP8)
    # [nt, n, j, p] -> [nt, p, j, n]
    return np.ascontiguousarray(
        wq.reshape(N // P, P, K // P, P).transpose(0, 3, 2, 1)
    )


def kernel(x, weight, weight_scale, bias, input_scale, output_scale):
    x = np.asarray(x, np.float32)
    weight = np.asarray(weight, np.float32)
    bias = np.asarray(bias, np.float32)
    B, S, K = x.shape
    N = weight.shape[0]
    M_total = B * S
    M = M_total // N_CORES
    MF = 512
    MB = M // MF
    KS = K // P
    NT = N // P

    si = float(np.asarray(input_scale, np.float64))
    sw = float(np.asarray(weight_scale, np.float64))
    os_ = float(np.asarray(output_scale, np.float64))
    inv2si = np.float32(1.0 / (2.0 * si))
    alpha = np.full((128, 1), 2.0 * si * sw / os_, np.float32)
    two_os = np.float32(2.0 * os_)
    bias2 = np.ascontiguousarray(
        (bias.astype(np.float64) / (2.0 * os_)).astype(np.float32)
        .reshape(N // P, P).T
    )

    # Host-side static quantize of x to the halved TRN-fp8 grid. Exact
    # per-tensor-static semantics: fp8e4(x/(2 s_in)) == e4m3fn(x/s_in)/2
    # for |x/s_in| <= 448, which holds by construction of input_scale
    # (clamp as a fallback when it doesn't).
    xf = x.reshape(M_total, K) * inv2si
    if np.abs(x).max() * float(inv2si) > 230.0:
        np.clip(xf, -224.0, 224.0, out=xf)
    xq = xf.astype(NP_FP8)  # [M_total, K]
    del xf

    wt = prep_weight(weight)

    in_maps = []
    for c in range(N_CORES):
        # qxt[mb, p, j, m] = xq[c*M + mb*MF + m, j*128 + p]
        qc = xq[c * M : (c + 1) * M].reshape(MB, MF, KS, P)
        in_maps.append({
            "qxt": np.ascontiguousarray(qc.transpose(0, 3, 2, 1)),
            "wt": wt,
            "bias2": bias2,
            "alpha": alpha,
        })

    nc = build(K, M, N, MF=MF)

    res = None
    last_exc = None
    for attempt in range(3):
        try:
            res = run_bass_kernel_spmd(nc, in_maps, core_ids=list(range(N_CORES)))
            break
        except Exception as e:  # transient NRT/device errors: retry
            last_exc = e
    if res is None:
        raise last_exc
    global LAST_RESULT
    LAST_RESULT = res

    out = np.empty((M_total, N), np.float32)
    for c in range(N_CORES):
        o8 = res.results[c]["out8"]  # [MB, NT, P, MF] fp8
        # out[m, n] = o8[mb, nt, p, mf] * 2os,  m=mb*MF+mf, n=nt*128+p
        oc = o8.astype(np.float32).transpose(0, 3, 1, 2).reshape(M, N)
        np.multiply(oc, two_os, out=out[c * M : (c + 1) * M])
    return out.reshape(B, S, N)


# revision 15
# speedup vs baseline: 1.0018x; 1.0018x over previous
"""FP8StaticLinear Trainium2 kernel.

out = requant_fp8(qdq_fp8(x, s_in) @ (w * s_w).T + bias, s_out)

Sharding: data-parallel over tokens (B*S=16384 -> 2048/core on 8 cores).
Device math: fp8e4 DoubleRow matmuls on the PE array. Both operands are
halved on entry so the OCP-e4m3fn grid (max 448) maps onto TRN fp8e4
(max 240) exactly; scales are folded into the epilogue and the host-side
pre/post conversions (x is pre-quantized to fp8 on the host, the output
leaves the device as fp8 and is expanded to f32*2os on the host — both
are exact given the static per-tensor scales).
"""

import numpy as np
import ml_dtypes

import concourse.bass as bass
import concourse.mybir as mybir
from concourse.tile import TileContext
from concourse.vector_clock import ScopedClock
from concourse.bass_utils import run_bass_kernel_spmd

FP8 = mybir.dt.float8e4
F32 = mybir.dt.float32
NP_FP8 = ml_dtypes.float8_e4m3  # TRN fp8e4 (max 240, has inf)

N_CORES = 8
P = 128


# ---------------------------------------------------------------------------
# Workaround: this walrus build rejects >1 sem-wait on the Tile tail Drain
# ("Too many sync wait commands"). Split the waits across single-wait drains.
def _drain_and_barrier(self, tick_clock, wait_clock):
    drain_inst = self.nc.sync.drain()
    wait_clock.add_sem_waits(
        drain_inst.ins, ScopedClock({None: tick_clock.global_clock})
    )
    w = list(drain_inst.ins.sync_info.on_wait)
    if len(w) > 1:
        drain_inst.ins.sync_info = mybir.SyncInfo(on_wait=[w[0]], on_update=[])
        for extra in w[1:]:
            d2 = self.nc.sync.drain()
            d2.ins.sync_info = mybir.SyncInfo(on_wait=[extra], on_update=[])
    self.nc.all_engine_barrier()
    assert self.sems is not None
    popped = self.nc._tile_sem_poison_stack.pop()
    assert popped is self._sem_poison
    self.nc.clear_and_free_semaphores(list(self.sems.allocated().values()))
    self.nc.all_engine_barrier()


TileContext._drain_and_barrier = _drain_and_barrier


def split_sync_waits(nc, max_waits=1):
    """Hoist extra sem-waits onto standalone EventSemaphore carriers.

    This walrus build's setupSyncWait rejects instructions carrying more
    than one sem-wait ("Too many sync wait commands"), so any instruction
    with N>1 waits becomes N-1 single-wait EventSemaphore instructions on
    the same engine followed by the original instruction with one wait.
    """
    n_new = 0
    for f in nc.m.functions:
        for blk in f.blocks:
            out = []
            changed = False
            for inst in blk.instructions:
                si = inst.sync_info
                w = list(si.on_wait) if si is not None else []
                if len(w) > max_waits:
                    upd = list(inst.sync_info.on_update)
                    for wi in w[max_waits:]:
                        es = mybir.InstEventSemaphore(
                            name=f"hoistw-{n_new}", ins=[], outs=[]
                        )
                        n_new += 1
                        es.engine = inst.engine
                        es.sync_info = mybir.SyncInfo(on_wait=[wi], on_update=[])
                        out.append(es)
                    inst.sync_info = mybir.SyncInfo(
                        on_wait=w[:max_waits], on_update=upd
                    )
                    changed = True
                out.append(inst)
            if changed:
                blk.instructions = out
    return nc
# ---------------------------------------------------------------------------


def build(K, M, N, MF=512, warm_mms=16):
    """One-core program: out8 = fp8 requant of (qx @ w.T)*alpha + bias2.

    DRAM inputs:
      qxt    [MB, 128, KS, MF] fp8   host-quantized x shard:
                     qxt[mb, p, j, m] = fp8(x[mb*MF+m, j*128+p] / (2*s_in))
      wt     [NT, 128, KS, 128] fp8  halved weight, tiled:
                     wt[nt, p, j, n] = fp8(w[nt*128+n, j*128+p] / 2)
      bias2  [N]  f32   bias / (2*s_out)
      alpha  [1, 1] f32  2*s_in*s_w/s_out
    Output:
      out8   [MB, NT, 128, MF] fp8   out8[mb, nt, p, m] = fp8(clamp(
                     (x@w.T + b)[mb*MF+m, nt*128+p] / (2*s_out), +-224))
    """
    KS = K // P          # k subtiles of 128
    JP = KS // 2         # DoubleRow pairs
    NT = N // P          # n tiles
    MB = M // MF         # m blocks
    AF = mybir.ActivationFunctionType
    OP = mybir.AluOpType

    nc = bass.Bass()
    qxt = nc.dram_tensor("qxt", [MB, P, KS, MF], FP8, kind="ExternalInput")
    wt = nc.dram_tensor("wt", [NT, P, KS, P], FP8, kind="ExternalInput")
    bias2_d = nc.dram_tensor("bias2", [P, N // P], F32, kind="ExternalInput")
    alpha_d = nc.dram_tensor("alpha", [P, 1], F32, kind="ExternalInput")
    out8 = nc.dram_tensor("out8", [MB, NT, P, MF], FP8, kind="ExternalOutput")

    with TileContext(nc) as tc:
        with (
            tc.tile_pool(name="consts", bufs=1) as consts,
            tc.tile_pool(name="wres", bufs=1) as wres,
            tc.tile_pool(name="qx", bufs=3) as qxp,
            tc.tile_pool(name="psum", bufs=8, space="PSUM") as psp,
            tc.tile_pool(name="epi", bufs=3) as epi,
            tc.tile_pool(name="q8", bufs=3) as q8p,
        ):
            # ---- PE clock warm-up: the HAM gate releases the 2.4 GHz
            # clock only after ~3.4us of sustained PE activity. Burn that
            # window on dummy matmuls while the first real weight/x DMAs
            # are still in flight, so the real stream starts at full rate.
            # The dummy operand tile is left almost entirely uninitialized
            # -- a full init (memset or DMA) would push the warm-up past
            # the point where real data lands. Tile only requires at least
            # one write to materialize the tile, so memset a single
            # element; the rest is garbage fp8, which is fine since the
            # target PSUM bank is overwritten (start=True) before its
            # first real use.
            zd = consts.tile([P, 2, MF], FP8)
            nc.gpsimd.memset(zd[:, 0:1, 0:1], 0.0)
            ps_dummy = psp.tile([P, MF], F32, tag="ps", name="psdummy")
            for _ in range(warm_mms):
                nc.tensor.matmul(
                    ps_dummy[:, 0:256], zd[:, :, 0:P], zd[:, :, 0:256],
                    start=True, stop=True,
                    perf_mode=mybir.MatmulPerfMode.DoubleRow,
                )

            # ---- weights: resident, one tile per nt. Warm-group tiles
            # stream in k-quarters ordered the way the k-outer warm loop
            # consumes them; the rest trail in halves. All weight + later
            # x-block DMAs ride the gpsimd SWDGE queue so the sync queue
            # stays clear for block-0 chunks and output stores.
            w_tiles = []

            def alloc_w(nt):
                w_nt = wres.tile([P, KS, P], FP8, tag=f"w{nt}", name=f"w{nt}")
                w_tiles.append(w_nt)
                return w_nt

            # constants FIRST on the queue: Tile waits on per-HW-queue DMA
            # completion counters in program order, so a late-emitted const
            # DMA would make every epilogue wait for all earlier DMAs
            alpha = consts.tile([P, 1], F32)
            nc.sync.dma_start(alpha[:], alpha_d[:, :])
            # bias2[p, nt] = bias[nt*128+p] / (2*os), transposed on host so
            # the DMA is contiguous (a strided rearrange costs ~4us of
            # descriptor generation on the critical path)
            bias2 = consts.tile([P, NT], F32)
            nc.sync.dma_start(bias2[:], bias2_d[:, :])

            # NW=7 warm groups + the dummy tile fill all 8 PSUM banks, and
            # the first steady group (nt=7) lands on the dummy bank, whose
            # matmuls finished long ago -- no epilogue wait at the
            # warm->steady boundary
            NW = min(7, NT)
            for g in range(NW):
                alloc_w(g)
            # block-0 x chunks on the sync queue (highest priority: the
            # PE is gated on the first two)
            qx_tiles = {}
            for mb in range(MB):
                qx_tiles[mb] = qxp.tile([P, KS, MF], FP8, tag="qx", name=f"qx{mb}")
            # block-0 x loads in quarter-blocks of 8 chunks (512 KB,
            # contiguous 4KB per-partition lines -> one cheap issue each);
            # the warm loop consumes 4 jj-rows per quarter, slower than
            # delivery
            for j0 in range(0, KS, 8):
                nc.sync.dma_start(
                    qx_tiles[0][:, j0 : j0 + 8, :],
                    qxt[0, :, j0 : j0 + 8, :],
                )
            # one DMA per weight tile (issue slots cost ~0.65us each
            # regardless of size, and SWDGE dep thresholds resolve several
            # issue-slots late anyway)
            for g in range(NW):
                nc.gpsimd.dma_start(w_tiles[g][:], wt[g, :, :, :])
            for nt in range(NW, NT):
                w_nt = alloc_w(nt)
                nc.gpsimd.dma_start(w_nt[:], wt[nt, :, :, :])
            # later x blocks trail on the same queue (pool WAR sems hold
            # them until the buffer frees; plenty of slack)
            CG = 8  # chunks per DMA for the later blocks
            for mb in range(1, MB):
                for j0 in range(0, KS, CG):
                    nc.gpsimd.dma_start(
                        qx_tiles[mb][:, j0 : j0 + CG, :],
                        qxt[mb, :, j0 : j0 + CG, :],
                    )

            def emit_mms(ps, nt, qx):
                for jj in range(JP):
                    nc.tensor.matmul(
                        ps[:],
                        w_tiles[nt][:, 2 * jj : 2 * jj + 2, :],
                        qx[:, 2 * jj : 2 * jj + 2, :],
                        start=(jj == 0),
                        stop=(jj == JP - 1),
                        perf_mode=mybir.MatmulPerfMode.DoubleRow,
                    )

            # epilogue: t = ps*alpha + bias/(2os); out8 = fp8(clamp t)
            def emit_epilogue(ps, nt, mb):
                t = epi.tile([P, MF], F32, tag="t", name="t")
                nc.scalar.activation(
                    t[:], ps[:], AF.Identity,
                    bias=bias2[:, nt : nt + 1], scale=alpha[:, 0:1],
                )
                q8 = q8p.tile([P, MF], FP8, tag="q8", name="q8")
                nc.vector.tensor_scalar(
                    q8[:], t[:], -224.0, 224.0, OP.max, OP.min
                )
                nc.sync.dma_start(out8[mb, nt, :, :], q8[:])

            # ---- main loop over m blocks ----
            for mb in range(MB):
                qx = qx_tiles[mb]
                if mb == 0:
                    # warm-up: first NW groups accumulate k-outer across NW
                    # psum banks, so the PE issues NW matmuls per arriving
                    # chunk pair instead of idling for the full qx0
                    ps_warm = [
                        psp.tile([P, MF], F32, tag="ps", name=f"psw{g}")
                        for g in range(NW)
                    ]
                    for jj in range(JP):
                        for g in range(NW):
                            nc.tensor.matmul(
                                ps_warm[g][:],
                                w_tiles[g][:, 2 * jj : 2 * jj + 2, :],
                                qx[:, 2 * jj : 2 * jj + 2, :],
                                start=(jj == 0),
                                stop=(jj == JP - 1),
                                perf_mode=mybir.MatmulPerfMode.DoubleRow,
                            )
                    for g in range(NW):
                        emit_epilogue(ps_warm[g], g, mb)
                    nt_range = list(range(NW, NT))
                else:
                    nt_range = list(range(NT))

                for nt in nt_range:
                    ps = psp.tile([P, MF], F32, tag="ps", name="ps")
                    emit_mms(ps, nt, qx)
                    emit_epilogue(ps, nt, mb)
    return split_sync_waits(nc)


def prep_weight(weight):
    """[N, K] f32 (e4m3fn-grid values) -> [NT, 128, KS, 128] TRN-fp8 of w/2."""
    N, K = weight.shape
    wq = (weight.astype(np.float32) * 0.5).astype(NP_FP8)
    # [nt, n, j, p] -> [nt, p, j, n]
    return np.ascontiguousarray(
        wq.reshape(N // P, P, K // P, P).transpose(0, 3, 2, 1)
    )


def kernel(x, weight, weight_scale, bias, input_scale, output_scale):
    x = np.asarray(x, np.float32)
    weight = np.asarray(weight, np.float32)
    bias = np.asarray(bias, np.float32)
    B, S, K = x.shape
    N = weight.shape[0]
    M_total = B * S
    M = M_total // N_CORES
    MF = 512
    MB = M // MF
    KS = K // P
    NT = N // P

    si = float(np.asarray(input_scale, np.float64))
    sw = float(np.asarray(weight_scale, np.float64))
    os_ = float(np.asarray(output_scale, np.float64))
    inv2si = np.float32(1.0 / (2.0 * si))
    alpha = np.full((128, 1), 2.0 * si * sw / os_, np.float32)
    two_os = np.float32(2.0 * os_)
    bias2 = np.ascontiguousarray(
        (bias.astype(np.float64) / (2.0 * os_)).astype(np.float32)
        .reshape(N // P, P).T
    )

    # Host-side static quantize of x to the halved TRN-fp8 grid. Exact
    # per-tensor-static semantics: fp8e4(x/(2 s_in)) == e4m3fn(x/s_in)/2
    # for |x/s_in| <= 448, which holds by construction of input_scale
    # (clamp as a fallback when it doesn't).
    xf = x.reshape(M_total, K) * inv2si
    if np.abs(x).max() * float(inv2si) > 230.0:
        np.clip(xf, -224.0, 224.0, out=xf)
    xq = xf.astype(NP_FP8)  # [M_total, K]
    del xf

    wt = prep_weight(weight)

    in_maps = []
    for c in range(N_CORES):
        # qxt[mb, p, j, m] = xq[c*M + mb*MF + m, j*128 + p]
        qc = xq[c * M : (c + 1) * M].reshape(MB, MF, KS, P)
        in_maps.append({
            "qxt": np.ascontiguousarray(qc.transpose(0, 3, 2, 1)),
            "wt": wt,
            "bias2": bias2,
            "alpha": alpha,
        })

    nc = build(K, M, N, MF=MF)

    res = None
    last_exc = None
    for attempt in range(3):
        try:
            res = run_bass_kernel_spmd(nc, in_maps, core_ids=list(range(N_CORES)))
            break
        except Exception as e:  # transient NRT/device errors: retry
            last_exc = e
    if res is None:
        raise last_exc
    global LAST_RESULT
    LAST_RESULT = res

    out = np.empty((M_total, N), np.float32)
    for c in range(N_CORES):
        o8 = res.results[c]["out8"]  # [MB, NT, P, MF] fp8
        # out[m, n] = o8[mb, nt, p, mf] * 2os,  m=mb*MF+mf, n=nt*128+p
        oc = o8.astype(np.float32).transpose(0, 3, 1, 2).reshape(M, N)
        np.multiply(oc, two_os, out=out[c * M : (c + 1) * M])
    return out.reshape(B, S, N)


# revision 16
# speedup vs baseline: 1.0040x; 1.0022x over previous
"""FP8StaticLinear Trainium2 kernel.

out = requant_fp8(qdq_fp8(x, s_in) @ (w * s_w).T + bias, s_out)

Sharding: data-parallel over tokens (B*S=16384 -> 2048/core on 8 cores).
Device math: fp8e4 DoubleRow matmuls on the PE array. Both operands are
halved on entry so the OCP-e4m3fn grid (max 448) maps onto TRN fp8e4
(max 240) exactly; scales are folded into the epilogue and the host-side
pre/post conversions (x is pre-quantized to fp8 on the host, the output
leaves the device as fp8 and is expanded to f32*2os on the host — both
are exact given the static per-tensor scales).
"""

import numpy as np
import ml_dtypes

import concourse.bass as bass
import concourse.mybir as mybir
from concourse.tile import TileContext
from concourse.vector_clock import ScopedClock
from concourse.bass_utils import run_bass_kernel_spmd

FP8 = mybir.dt.float8e4
F32 = mybir.dt.float32
NP_FP8 = ml_dtypes.float8_e4m3  # TRN fp8e4 (max 240, has inf)

N_CORES = 8
P = 128


# ---------------------------------------------------------------------------
# Workaround: this walrus build rejects >1 sem-wait on the Tile tail Drain
# ("Too many sync wait commands"). Split the waits across single-wait drains.
def _drain_and_barrier(self, tick_clock, wait_clock):
    drain_inst = self.nc.sync.drain()
    wait_clock.add_sem_waits(
        drain_inst.ins, ScopedClock({None: tick_clock.global_clock})
    )
    w = list(drain_inst.ins.sync_info.on_wait)
    if len(w) > 1:
        drain_inst.ins.sync_info = mybir.SyncInfo(on_wait=[w[0]], on_update=[])
        for extra in w[1:]:
            d2 = self.nc.sync.drain()
            d2.ins.sync_info = mybir.SyncInfo(on_wait=[extra], on_update=[])
    self.nc.all_engine_barrier()
    assert self.sems is not None
    popped = self.nc._tile_sem_poison_stack.pop()
    assert popped is self._sem_poison
    self.nc.clear_and_free_semaphores(list(self.sems.allocated().values()))
    self.nc.all_engine_barrier()


TileContext._drain_and_barrier = _drain_and_barrier


def split_sync_waits(nc, max_waits=1):
    """Hoist extra sem-waits onto standalone EventSemaphore carriers.

    This walrus build's setupSyncWait rejects instructions carrying more
    than one sem-wait ("Too many sync wait commands"), so any instruction
    with N>1 waits becomes N-1 single-wait EventSemaphore instructions on
    the same engine followed by the original instruction with one wait.
    """
    n_new = 0
    for f in nc.m.functions:
        for blk in f.blocks:
            out = []
            changed = False
            for inst in blk.instructions:
                si = inst.sync_info
                w = list(si.on_wait) if si is not None else []
                if len(w) > max_waits:
                    upd = list(inst.sync_info.on_update)
                    for wi in w[max_waits:]:
                        es = mybir.InstEventSemaphore(
                            name=f"hoistw-{n_new}", ins=[], outs=[]
                        )
                        n_new += 1
                        es.engine = inst.engine
                        es.sync_info = mybir.SyncInfo(on_wait=[wi], on_update=[])
                        out.append(es)
                    inst.sync_info = mybir.SyncInfo(
                        on_wait=w[:max_waits], on_update=upd
                    )
                    changed = True
                out.append(inst)
            if changed:
                blk.instructions = out
    return nc
# ---------------------------------------------------------------------------


def build(K, M, N, MF=512, warm_mms=40):
    """One-core program: out8 = fp8 requant of (qx @ w.T)*alpha + bias2.

    DRAM inputs:
      qxt    [MB, 128, KS, MF] fp8   host-quantized x shard:
                     qxt[mb, p, j, m] = fp8(x[mb*MF+m, j*128+p] / (2*s_in))
      wt     [NT, 128, KS, 128] fp8  halved weight, tiled:
                     wt[nt, p, j, n] = fp8(w[nt*128+n, j*128+p] / 2)
      bias2  [N]  f32   bias / (2*s_out)
      alpha  [1, 1] f32  2*s_in*s_w/s_out
    Output:
      out8   [MB, NT, 128, MF] fp8   out8[mb, nt, p, m] = fp8(clamp(
                     (x@w.T + b)[mb*MF+m, nt*128+p] / (2*s_out), +-224))
    """
    KS = K // P          # k subtiles of 128
    JP = KS // 2         # DoubleRow pairs
    NT = N // P          # n tiles
    MB = M // MF         # m blocks
    AF = mybir.ActivationFunctionType
    OP = mybir.AluOpType

    nc = bass.Bass()
    qxt = nc.dram_tensor("qxt", [MB, P, KS, MF], FP8, kind="ExternalInput")
    wt = nc.dram_tensor("wt", [NT, P, KS, P], FP8, kind="ExternalInput")
    bias2_d = nc.dram_tensor("bias2", [P, N // P], F32, kind="ExternalInput")
    alpha_d = nc.dram_tensor("alpha", [P, 1], F32, kind="ExternalInput")
    out8 = nc.dram_tensor("out8", [MB, NT, P, MF], FP8, kind="ExternalOutput")

    with TileContext(nc) as tc:
        with (
            tc.tile_pool(name="consts", bufs=1) as consts,
            tc.tile_pool(name="wres", bufs=1) as wres,
            tc.tile_pool(name="qx", bufs=3) as qxp,
            tc.tile_pool(name="psum", bufs=8, space="PSUM") as psp,
            tc.tile_pool(name="epi", bufs=3) as epi,
            tc.tile_pool(name="q8", bufs=3) as q8p,
        ):
            # ---- PE clock warm-up: the HAM gate releases the 2.4 GHz
            # clock only after ~3.4us of sustained PE activity. Burn that
            # window on dummy matmuls while the first real weight/x DMAs
            # are still in flight, so the real stream starts at full rate.
            # The dummy operand tile is left almost entirely uninitialized
            # -- a full init (memset or DMA) would push the warm-up past
            # the point where real data lands. Tile only requires at least
            # one write to materialize the tile, so memset a single
            # element; the rest is garbage fp8, which is fine since the
            # target PSUM bank is overwritten (start=True) before its
            # first real use.
            zd = consts.tile([P, 2, MF], FP8)
            nc.gpsimd.memset(zd[:, 0:1, 0:1], 0.0)
            ps_dummy = psp.tile([P, MF], F32, tag="ps", name="psdummy")
            for _ in range(warm_mms):
                nc.tensor.matmul(
                    ps_dummy[:, 0:256], zd[:, :, 0:P], zd[:, :, 0:256],
                    start=True, stop=True,
                    perf_mode=mybir.MatmulPerfMode.DoubleRow,
                )

            # ---- weights: resident, one tile per nt. Warm-group tiles
            # stream in k-quarters ordered the way the k-outer warm loop
            # consumes them; the rest trail in halves. All weight + later
            # x-block DMAs ride the gpsimd SWDGE queue so the sync queue
            # stays clear for block-0 chunks and output stores.
            w_tiles = []

            def alloc_w(nt):
                w_nt = wres.tile([P, KS, P], FP8, tag=f"w{nt}", name=f"w{nt}")
                w_tiles.append(w_nt)
                return w_nt

            # constants FIRST on the queue: Tile waits on per-HW-queue DMA
            # completion counters in program order, so a late-emitted const
            # DMA would make every epilogue wait for all earlier DMAs
            alpha = consts.tile([P, 1], F32)
            nc.sync.dma_start(alpha[:], alpha_d[:, :])
            # bias2[p, nt] = bias[nt*128+p] / (2*os), transposed on host so
            # the DMA is contiguous (a strided rearrange costs ~4us of
            # descriptor generation on the critical path)
            bias2 = consts.tile([P, NT], F32)
            nc.sync.dma_start(bias2[:], bias2_d[:, :])

            # NW=7 warm groups + the dummy tile fill all 8 PSUM banks, and
            # the first steady group (nt=7) lands on the dummy bank, whose
            # matmuls finished long ago -- no epilogue wait at the
            # warm->steady boundary
            NW = min(7, NT)
            for g in range(NW):
                alloc_w(g)
            # block-0 x chunks on the sync queue (highest priority: the
            # PE is gated on the first two)
            qx_tiles = {}
            for mb in range(MB):
                qx_tiles[mb] = qxp.tile([P, KS, MF], FP8, tag="qx", name=f"qx{mb}")
            # block-0 x loads in quarter-blocks of 8 chunks (512 KB,
            # contiguous 4KB per-partition lines -> one cheap issue each);
            # the warm loop consumes 4 jj-rows per quarter, slower than
            # delivery
            for j0 in range(0, KS, 8):
                nc.sync.dma_start(
                    qx_tiles[0][:, j0 : j0 + 8, :],
                    qxt[0, :, j0 : j0 + 8, :],
                )
            # one DMA per weight tile (issue slots cost ~0.65us each
            # regardless of size, and SWDGE dep thresholds resolve several
            # issue-slots late anyway)
            for g in range(NW):
                nc.gpsimd.dma_start(w_tiles[g][:], wt[g, :, :, :])
            for nt in range(NW, NT):
                w_nt = alloc_w(nt)
                nc.gpsimd.dma_start(w_nt[:], wt[nt, :, :, :])
            # later x blocks trail on the same queue (pool WAR sems hold
            # them until the buffer frees; plenty of slack)
            CG = 8  # chunks per DMA for the later blocks
            for mb in range(1, MB):
                for j0 in range(0, KS, CG):
                    nc.gpsimd.dma_start(
                        qx_tiles[mb][:, j0 : j0 + CG, :],
                        qxt[mb, :, j0 : j0 + CG, :],
                    )

            def emit_mms(ps, nt, qx):
                for jj in range(JP):
                    nc.tensor.matmul(
                        ps[:],
                        w_tiles[nt][:, 2 * jj : 2 * jj + 2, :],
                        qx[:, 2 * jj : 2 * jj + 2, :],
                        start=(jj == 0),
                        stop=(jj == JP - 1),
                        perf_mode=mybir.MatmulPerfMode.DoubleRow,
                    )

            # epilogue: t = ps*alpha + bias/(2os); out8 = fp8(clamp t)
            def emit_epilogue(ps, nt, mb):
                t = epi.tile([P, MF], F32, tag="t", name="t")
                nc.scalar.activation(
                    t[:], ps[:], AF.Identity,
                    bias=bias2[:, nt : nt + 1], scale=alpha[:, 0:1],
                )
                q8 = q8p.tile([P, MF], FP8, tag="q8", name="q8")
                nc.vector.tensor_scalar(
                    q8[:], t[:], -224.0, 224.0, OP.max, OP.min
                )
                nc.sync.dma_start(out8[mb, nt, :, :], q8[:])

            # ---- main loop over m blocks ----
            for mb in range(MB):
                qx = qx_tiles[mb]
                if mb == 0:
                    # warm-up: first NW groups accumulate k-outer across NW
                    # psum banks, so the PE issues NW matmuls per arriving
                    # chunk pair instead of idling for the full qx0
                    ps_warm = [
                        psp.tile([P, MF], F32, tag="ps", name=f"psw{g}")
                        for g in range(NW)
                    ]
                    for jj in range(JP):
                        for g in range(NW):
                            nc.tensor.matmul(
                                ps_warm[g][:],
                                w_tiles[g][:, 2 * jj : 2 * jj + 2, :],
                                qx[:, 2 * jj : 2 * jj + 2, :],
                                start=(jj == 0),
                                stop=(jj == JP - 1),
                                perf_mode=mybir.MatmulPerfMode.DoubleRow,
                            )
                    for g in range(NW):
                        emit_epilogue(ps_warm[g], g, mb)
                    nt_range = list(range(NW, NT))
                else:
                    nt_range = list(range(NT))

                for nt in nt_range:
                    ps = psp.tile([P, MF], F32, tag="ps", name="ps")
                    emit_mms(ps, nt, qx)
                    emit_epilogue(ps, nt, mb)
    return split_sync_waits(nc)


def prep_weight(weight):
    """[N, K] f32 (e4m3fn-grid values) -> [NT, 128, KS, 128] TRN-fp8 of w/2."""
    N, K = weight.shape
    wq = (weight.astype(np.float32) * 0.5).astype(NP_FP8)
    # [nt, n, j, p] -> [nt, p, j, n]
    return np.ascontiguousarray(
        wq.reshape(N // P, P, K // P, P).transpose(0, 3, 2, 1)
    )


def kernel(x, weight, weight_scale, bias, input_scale, output_scale):
    x = np.asarray(x, np.float32)
    weight = np.asarray(weight, np.float32)
    bias = np.asarray(bias, np.float32)
    B, S, K = x.shape
    N = weight.shape[0]
    M_total = B * S
    M = M_total // N_CORES
    MF = 512
    MB = M // MF
    KS = K // P
    NT = N // P

    si = float(np.asarray(input_scale, np.float64))
    sw = float(np.asarray(weight_scale, np.float64))
    os_ = float(np.asarray(output_scale, np.float64))
    inv2si = np.float32(1.0 / (2.0 * si))
    alpha = np.full((128, 1), 2.0 * si * sw / os_, np.float32)
    two_os = np.float32(2.0 * os_)
    bias2 = np.ascontiguousarray(
        (bias.astype(np.float64) / (2.0 * os_)).astype(np.float32)
        .reshape(N // P, P).T
    )

    # Host-side static quantize of x to the halved TRN-fp8 grid. Exact
    # per-tensor-static semantics: fp8e4(x/(2 s_in)) == e4m3fn(x/s_in)/2
    # for |x/s_in| <= 448, which holds by construction of input_scale
    # (clamp as a fallback when it doesn't).
    xf = x.reshape(M_total, K) * inv2si
    if np.abs(x).max() * float(inv2si) > 230.0:
        np.clip(xf, -224.0, 224.0, out=xf)
    xq = xf.astype(NP_FP8)  # [M_total, K]
    del xf

    wt = prep_weight(weight)

    in_maps = []
    for c in range(N_CORES):
        # qxt[mb, p, j, m] = xq[c*M + mb*MF + m, j*128 + p]
        qc = xq[c * M : (c + 1) * M].reshape(MB, MF, KS, P)
        in_maps.append({
            "qxt": np.ascontiguousarray(qc.transpose(0, 3, 2, 1)),
            "wt": wt,
            "bias2": bias2,
            "alpha": alpha,
        })

    nc = build(K, M, N, MF=MF)

    res = None
    last_exc = None
    for attempt in range(3):
        try:
            res = run_bass_kernel_spmd(nc, in_maps, core_ids=list(range(N_CORES)))
            break
        except Exception as e:  # transient NRT/device errors: retry
            last_exc = e
    if res is None:
        raise last_exc
    global LAST_RESULT
    LAST_RESULT = res

    out = np.empty((M_total, N), np.float32)
    for c in range(N_CORES):
        o8 = res.results[c]["out8"]  # [MB, NT, P, MF] fp8
        # out[m, n] = o8[mb, nt, p, mf] * 2os,  m=mb*MF+mf, n=nt*128+p
        oc = o8.astype(np.float32).transpose(0, 3, 1, 2).reshape(M, N)
        np.multiply(oc, two_os, out=out[c * M : (c + 1) * M])
    return out.reshape(B, S, N)
